# revision 30
# baseline (speedup 1.0000x reference)
"""Trainium2 Bass kernel for nn_Attention_523986010726.

Dense GQA attention layer (B=2, S=2048, D=4096, 32 q-heads / 8 kv-heads,
head_dim=128, RoPE, causal mask, fused QKV+SDPA+output projection).

Sharding (per spec hint): tensor-parallel across heads over 8 NeuronCores.
Each core owns 1 kv-head + its 4 q-heads: Wq/Wk/Wv column-sharded,
Wo row-sharded.  Per-core partial outputs are combined with an on-chip
ReduceScatter (8 cores, chunked per 512-token tile so the collective
overlaps compute); each core ends up with a 512-row slice of the
(DIM x B*S) transposed output, which the host reassembles.

Device dataflow (everything "transposed": feature dims on SBUF partitions):
  xT[d,t] (host-pretransposed, bf16) --matmul--> Q^T/K^T/V^T per t-tile
  RoPE applied in [hd, t] layout (rotate-half via SBUF->SBUF DMA partition
  shift; 1/sqrt(hd) folded into K's cos/sin tables)
  S^T[k,t] = K^T.T @ Q^T per 128-k-chunk; causal mask added on diagonal
  band tiles; P^T = exp(S^T) on ScalarE (no max-subtraction: |scores|<~18)
  out^T[hd,t] += V_chunk.T @ P^T accumulated in PSUM; softmax denominators
  via a ones-vector matmul on the same P^T chunks; normalization delayed
  to after AV (everything is linear in the k-sum), applied as
  out^T * broadcast(1/den) where the broadcast across partitions is a
  rank-1 matmul.
  O-proj: partial^T[dim,t] += WoT_chunk.T @ attn^T, DMA'd to DRAM and
  ReduceScattered across the 8 cores.
"""

import os
import sys
from contextlib import ExitStack

sys.path.insert(0, "/opt/trn_rl_repo")

import numpy as np
import ml_dtypes

B, S, DIM = 2, 2048, 4096
HQ, HKV, HD = 32, 8, 128
NCORES = 8
EH = HQ // NCORES          # q-heads per core (4)
E = EH * HD                # per-core q-projection width (512)
TT = 512                   # token tile (matmul moving free dim)

BF16 = ml_dtypes.bfloat16


def build_module(D=DIM, S_=S, TT_=TT):
    """Build the per-core Bass module (identical on all cores; per-core
    weight slices arrive as input values)."""
    import concourse.bass as bass  # noqa: F401
    import concourse.mybir as mybir
    import concourse.tile as tile
    from concourse import bacc

    f32 = mybir.dt.float32
    bf16 = mybir.dt.bfloat16
    AF = mybir.ActivationFunctionType

    DC = D // 128            # contraction chunks for projections
    NT_B = S_ // TT_         # t-tiles per batch
    NT = B * NT_B            # total t-tiles
    NPAT = TT_ // 128        # diagonal mask patterns
    KCB = S_ // 128          # k-chunks per batch
    DQ = 8 if DC % 8 == 0 else DC   # xT streaming sub-block (d-chunks)
    NDQ = DC // DQ

    nc = bacc.Bacc(num_devices=NCORES)

    xT = nc.dram_tensor("xT", [D, B * S_], bf16, kind="ExternalInput")
    wqT = nc.dram_tensor("wqT", [D, E], bf16, kind="ExternalInput")
    wkT = nc.dram_tensor("wkT", [D, HD], bf16, kind="ExternalInput")
    wvT = nc.dram_tensor("wvT", [D, HD], bf16, kind="ExternalInput")
    woT = nc.dram_tensor("woT", [E, D], bf16, kind="ExternalInput")
    bqc = nc.dram_tensor("bqc", [128, EH], f32, kind="ExternalInput")
    bkc = nc.dram_tensor("bkc", [128, 1], f32, kind="ExternalInput")
    bvc = nc.dram_tensor("bvc", [128, 1], f32, kind="ExternalInput")
    cosq = nc.dram_tensor("cosq", [128, S_], bf16, kind="ExternalInput")
    sinq = nc.dram_tensor("sinq", [128, S_], bf16, kind="ExternalInput")
    cosk = nc.dram_tensor("cosk", [128, S_], bf16, kind="ExternalInput")
    sink = nc.dram_tensor("sink", [128, S_], bf16, kind="ExternalInput")
    mpat = nc.dram_tensor("mpat", [128, NPAT, TT_], f32, kind="ExternalInput")
    idbf = nc.dram_tensor("idbf", [128, 128], bf16, kind="ExternalInput")
    y = nc.dram_tensor("y", [D // NCORES, B * S_], f32, kind="ExternalOutput")

    partials = [nc.dram_tensor(f"partial_{t}", [D, TT_], f32) for t in range(NT)]
    rsouts = [nc.dram_tensor(f"rsout_{t}", [D // NCORES, TT_], f32) for t in range(NT)]

    with tile.TileContext(nc) as tc, ExitStack() as ctx:
        const = ctx.enter_context(tc.tile_pool(name="const", bufs=1))
        xpool = ctx.enter_context(tc.tile_pool(name="xpool", bufs=NDQ + 1))
        rpool = ctx.enter_context(tc.tile_pool(name="rpool", bufs=3))
        qpool = ctx.enter_context(tc.tile_pool(name="qpool", bufs=2))
        ppool = ctx.enter_context(tc.tile_pool(name="ppool", bufs=4))
        apool = ctx.enter_context(tc.tile_pool(name="apool", bufs=2))
        opool = ctx.enter_context(tc.tile_pool(name="opool", bufs=3))
        spool = ctx.enter_context(tc.tile_pool(name="spool", bufs=2))

        # One shared PSUM pool: every phase can use all 8 banks, decoupling
        # the PE from the (slower) PSUM->SBUF drain engines.
        psum = ctx.enter_context(tc.tile_pool(name="psum", bufs=8, space="PSUM"))

        # ---- resident constants ----
        wq_sb = const.tile([128, DC, E], bf16)
        nc.sync.dma_start(wq_sb[:], wqT.ap().rearrange("(o p) e -> p o e", p=128))
        wk_sb = const.tile([128, DC, HD], bf16)
        nc.sync.dma_start(wk_sb[:], wkT.ap().rearrange("(o p) e -> p o e", p=128))
        wv_sb = const.tile([128, DC, HD], bf16)
        nc.sync.dma_start(wv_sb[:], wvT.ap().rearrange("(o p) e -> p o e", p=128))
        wo_sb = const.tile([128, EH, D], bf16)
        nc.sync.dma_start(wo_sb[:], woT.ap().rearrange("(o p) d -> p o d", p=128))
        bq_sb = const.tile([128, EH], f32)
        nc.sync.dma_start(bq_sb[:], bqc.ap())
        bk_sb = const.tile([128, 1], f32)
        nc.sync.dma_start(bk_sb[:], bkc.ap())
        bv_sb = const.tile([128, 1], f32)
        nc.sync.dma_start(bv_sb[:], bvc.ap())
        cosq_sb = const.tile([128, S_], bf16)
        nc.sync.dma_start(cosq_sb[:], cosq.ap())
        sinq_sb = const.tile([128, S_], bf16)
        nc.sync.dma_start(sinq_sb[:], sinq.ap())
        cosk_sb = const.tile([128, S_], bf16)
        nc.sync.dma_start(cosk_sb[:], cosk.ap())
        sink_sb = const.tile([128, S_], bf16)
        nc.sync.dma_start(sink_sb[:], sink.ap())
        mp_sb = const.tile([128, NPAT, TT_], f32)
        nc.sync.dma_start(mp_sb[:], mpat.ap())
        id_sb = const.tile([128, 128], bf16)
        nc.sync.dma_start(id_sb[:], idbf.ap())
        ones_col = const.tile([128, 1], bf16)
        nc.vector.memset(ones_col[:], 1.0)
        ones_row = const.tile([1, 128], f32)
        nc.vector.memset(ones_row[:], 1.0)

        # persistent K^T / V accumulation buffers (filled tile-by-tile)
        kT_sb = const.tile([128, B, S_], bf16)
        v_sb = const.tile([128, B, KCB, HD], bf16)

        xT_r = xT.ap().rearrange("(o p) t -> p o t", p=128)

        def rope(dst, src_f, cos_sb, sin_sb, t0):
            """dst[hd,t] = src*cos + rotate_half(src)*sin  (sin sign-folded).

            src_f: [128, TT] bf16 SBUF tile (pre-RoPE projection incl bias).
            """
            ssh = rpool.tile([128, TT_], bf16, tag="ssh")
            # partition rotate-by-64 via SBUF->SBUF DMA
            nc.sync.dma_start(ssh[0:64, :], src_f[64:128, :])
            nc.sync.dma_start(ssh[64:128, :], src_f[0:64, :])
            t1 = rpool.tile([128, TT_], bf16, tag="t1")
            nc.vector.tensor_mul(out=t1[:], in0=src_f[:], in1=cos_sb[:, t0:t0 + TT_])
            t2 = rpool.tile([128, TT_], bf16, tag="t2")
            nc.vector.tensor_mul(out=t2[:], in0=ssh[:], in1=sin_sb[:, t0:t0 + TT_])
            nc.vector.tensor_add(out=dst, in0=t1[:], in1=t2[:])

        for tt in range(NT):
            b = tt // NT_B
            t0 = (tt % NT_B) * TT_
            g0 = tt * TT_             # global token offset

            # ---- stream x^T block for this t-tile ----
            xq = []
            for qq in range(NDQ):
                xt_q = xpool.tile([128, DQ, TT_], bf16, tag="xt")
                nc.sync.dma_start(
                    xt_q[:], xT_r[:, qq * DQ:(qq + 1) * DQ, g0:g0 + TT_]
                )
                xq.append(xt_q)

            def proj_matmuls(ps, w_sb, esl):
                for dc in range(DC):
                    nc.tensor.matmul(
                        ps[:],
                        lhsT=w_sb[:, dc, esl],
                        rhs=xq[dc // DQ][:, dc % DQ, :],
                        start=(dc == 0),
                        stop=(dc == DC - 1),
                    )

            # ---- Q projection + RoPE ----
            q_blk = qpool.tile([128, EH, TT_], bf16)
            for e in range(EH):
                ps_q = psum.tile([128, TT_], f32, tag="ps")
                proj_matmuls(ps_q, wq_sb, slice(e * 128, (e + 1) * 128))
                qf = rpool.tile([128, TT_], bf16, tag="projf")
                nc.scalar.add(qf[:], ps_q[:], bq_sb[:, e:e + 1])
                rope(q_blk[:, e, :], qf, cosq_sb, sinq_sb, t0)

            # ---- K projection + RoPE (pre-scaled trig) ----
            ps_k = psum.tile([128, TT_], f32, tag="ps")
            proj_matmuls(ps_k, wk_sb, slice(0, HD))
            kf = rpool.tile([128, TT_], bf16, tag="projf")
            nc.scalar.add(kf[:], ps_k[:], bk_sb[:, 0:1])
            rope(kT_sb[:, b, t0:t0 + TT_], kf, cosk_sb, sink_sb, t0)

            # ---- V projection (V^T then transpose to natural [k, hd]) ----
            ps_v = psum.tile([128, TT_], f32, tag="ps")
            proj_matmuls(ps_v, wv_sb, slice(0, HD))
            vf = rpool.tile([128, TT_], bf16, tag="projf")
            nc.scalar.add(vf[:], ps_v[:], bv_sb[:, 0:1])
            for j in range(TT_ // 128):
                ps_t = psum.tile([128, 128], bf16, tag="ps")
                nc.tensor.transpose(ps_t[:], vf[:, j * 128:(j + 1) * 128], id_sb[:])
                nc.vector.tensor_copy(
                    out=v_sb[:, b, t0 // 128 + j, :], in_=ps_t[:]
                )

            # ---- attention for this query tile ----
            # Chunk order: interleave the 4 diagonal (masked) chunks between
            # full chunks so their extra DVE mask-add overlaps PE streaming.
            # The chunk loop is software-pipelined depth 2: AV/den matmuls for
            # chunk i issue after S^T/exp of chunk i+2, so the PE never waits
            # on ScalarE's exp.
            nkc = (t0 + TT_) // 128
            diag = list(range(t0 // 128, nkc))
            full = list(range(t0 // 128))
            order = []
            if full:
                stride = max(1, len(full) // len(diag))
                fi = 0
                for d_ in diag:
                    order.append(d_)
                    order.extend(full[fi:fi + stride])
                    fi += stride
                order.extend(full[fi:])
            else:
                order = diag
            assert sorted(order) == list(range(nkc))

            att = apool.tile([128, EH, TT_], bf16)
            pending = None  # (h, av, rec) epilogue deferred one head

            def epilogue(h_, av_, rec_):
                # broadcast 1/den across partitions (rank-1 matmul) and
                # normalize; rec_ was computed during the next head's chunk
                # loop so the bcast matmul never stalls the PE.
                bc = psum.tile([128, TT_], f32, tag="ps")
                nc.tensor.matmul(
                    bc[:], lhsT=ones_row[:], rhs=rec_[:], start=True, stop=True
                )
                avs = spool.tile([128, TT_], f32, tag="avs")
                nc.scalar.copy(avs[:], av_[:])
                nc.vector.tensor_mul(out=att[:, h_, :], in0=avs[:], in1=bc[:])

            for h in range(EH):
                av = psum.tile([128, TT_], f32, tag="ps")
                den = psum.tile([1, TT_], f32, tag="ps")
                DEPTH = 2
                pts = {}

                def av_den(i, h=h, av=av, den=den):
                    kc = order[i]
                    pt = pts.pop(i)
                    nc.tensor.matmul(
                        av[:], lhsT=v_sb[:, b, kc, :], rhs=pt[:],
                        start=(i == 0), stop=(i == nkc - 1),
                    )
                    nc.tensor.matmul(
                        den[:], lhsT=ones_col[:], rhs=pt[:],
                        start=(i == 0), stop=(i == nkc - 1),
                    )

                for i, kc in enumerate(order):
                    st = psum.tile([128, TT_], f32, tag="ps")
                    nc.tensor.matmul(
                        st[:],
                        lhsT=kT_sb[:, b, kc * 128:(kc + 1) * 128],
                        rhs=q_blk[:, h, :],
                        start=True,
                        stop=True,
                    )
                    d = kc * 128 - t0
                    if d >= 0:  # diagonal band: apply causal mask pattern
                        nc.vector.tensor_add(
                            out=st[:], in0=st[:], in1=mp_sb[:, d // 128, :]
                        )
                    pt = ppool.tile([128, TT_], bf16, tag="pt")
                    nc.scalar.activation(pt[:], st[:], AF.Exp)
                    pts[i] = pt
                    if i >= DEPTH:
                        av_den(i - DEPTH)
                for i in range(max(0, nkc - DEPTH), nkc):
                    av_den(i)

                rec = spool.tile([1, TT_], f32, tag="rec")
                nc.vector.reciprocal(rec[:], den[:])
                if pending is not None:
                    epilogue(*pending)
                pending = (h, av, rec)
            epilogue(*pending)

            # ---- output projection (partial, transposed) ----
            # PSUM->SBUF staging on DVE (keeps ScalarE free for next tile's
            # exps); partial writes on the ACT HWDGE queue so the SP queue
            # only carries latency-critical loads.
            for dt in range(DC):
                po = psum.tile([128, TT_], f32, tag="ps")
                for c in range(EH):
                    nc.tensor.matmul(
                        po[:],
                        lhsT=wo_sb[:, c, dt * 128:(dt + 1) * 128],
                        rhs=att[:, c, :],
                        start=(c == 0),
                        stop=(c == EH - 1),
                    )
                osb = opool.tile([128, TT_], f32, tag="osb")
                nc.scalar.copy(osb[:], po[:])
                nc.scalar.dma_start(
                    partials[tt].ap()[dt * 128:(dt + 1) * 128, :], osb[:]
                )

            # ---- chunked reduce-scatter + final copy ----
            import concourse.mybir as mybir_  # noqa: PLC0415
            nc.gpsimd.collective_compute(
                "ReduceScatter",
                mybir_.AluOpType.add,
                replica_groups=[list(range(NCORES))],
                ins=[partials[tt].ap().opt()],
                outs=[rsouts[tt].ap().opt()],
            )
            nc.gpsimd.dma_start(y.ap()[:, g0:g0 + TT_], rsouts[tt].ap())

    nc.finalize()
    return nc


def _prep_in_maps(x, cos, sin, Wq, bq, Wk, bk, Wv, bv, Wo, mask, D, S_, TT_):
    """Host-side sharding/prep: transpose+cast per-core operand slices."""
    NPAT = TT_ // 128
    scaling = np.float32(1.0 / np.sqrt(HD))

    xT = np.ascontiguousarray(
        x.reshape(B * S_, D).T.astype(BF16)
    )  # [D, B*S]
    cosT = cos.T.astype(np.float32)            # [HD, S]
    sinT = sin.T.astype(np.float32)
    sgn = np.ones((HD, 1), np.float32)
    sgn[: HD // 2] = -1.0
    cosq_h = np.ascontiguousarray(cosT.astype(BF16))
    sinq_h = np.ascontiguousarray((sinT * sgn).astype(BF16))
    cosk_h = np.ascontiguousarray((cosT * scaling).astype(BF16))
    sink_h = np.ascontiguousarray((sinT * sgn * scaling).astype(BF16))

    # causal mask diagonal patterns from the mask input:
    # mpat[p, i, f] = clip(mask[f, i*128 + p], -100, 0)
    m = np.asarray(mask).reshape(np.asarray(mask).shape[-2], -1)[:TT_, :TT_]
    mp = np.clip(m.T, -100.0, 0.0).astype(np.float32)        # [k, t]
    mpat = np.ascontiguousarray(
        mp.reshape(NPAT, 128, TT_).transpose(1, 0, 2)
    )  # [p, i, f]

    idbf = np.eye(128, dtype=BF16)

    in_maps = []
    for c in range(NCORES):
        e0 = c * E
        k0 = c * HD
        in_maps.append({
            "xT": xT,
            "wqT": np.ascontiguousarray(Wq[e0:e0 + E].T.astype(BF16)),
            "wkT": np.ascontiguousarray(Wk[k0:k0 + HD].T.astype(BF16)),
            "wvT": np.ascontiguousarray(Wv[k0:k0 + HD].T.astype(BF16)),
            "woT": np.ascontiguousarray(Wo[:, e0:e0 + E].T.astype(BF16)),
            "bqc": np.ascontiguousarray(
                bq[e0:e0 + E].reshape(EH, 128).T.astype(np.float32)
            ),
            "bkc": bk[k0:k0 + HD].reshape(HD, 1).astype(np.float32),
            "bvc": bv[k0:k0 + HD].reshape(HD, 1).astype(np.float32),
            "cosq": cosq_h,
            "sinq": sinq_h,
            "cosk": cosk_h,
            "sink": sink_h,
            "mpat": mpat,
            "idbf": idbf,
        })
    return in_maps


_NC_CACHE = {}


def _install_ntff_hook():
    """Register the axon NTFF profiling hook that this image's antenv lacks
    (used only when KERNEL_TRACE=1; grading runs without it)."""
    import types

    try:
        import antenv
        if "antenv.axon_hooks" not in sys.modules:
            mod = types.ModuleType("antenv.axon_hooks")
            state = {"hook": None}
            mod.set_axon_ntff_profile_hook = lambda h: state.__setitem__("hook", h)
            mod.get_axon_ntff_profile_hook = lambda: state["hook"]
            sys.modules["antenv.axon_hooks"] = mod
            antenv.axon_hooks = mod
        import antenv.axon_hooks as ah
        if ah.get_axon_ntff_profile_hook() is None:
            from trn_agent_boot.trn_boot import _ntff_profile_via_ctypes
            ah.set_axon_ntff_profile_hook(
                _ntff_profile_via_ctypes("/opt/axon/libaxon_pjrt.so")
            )
        return True
    except Exception as e:  # pragma: no cover - best effort
        print(f"NTFF hook install failed ({e}); running without trace")
        return False


def kernel(**inputs):
    x = np.asarray(inputs["x"], np.float32)
    cos = np.asarray(inputs["cos"], np.float32)
    sin = np.asarray(inputs["sin"], np.float32)
    Wq = np.asarray(inputs["Wq"], np.float32)
    bq = np.asarray(inputs["bq"], np.float32)
    Wk = np.asarray(inputs["Wk"], np.float32)
    bk = np.asarray(inputs["bk"], np.float32)
    Wv = np.asarray(inputs["Wv"], np.float32)
    bv = np.asarray(inputs["bv"], np.float32)
    Wo = np.asarray(inputs["Wo"], np.float32)
    mask = np.asarray(inputs["mask"], np.float32)
    start_pos = int(inputs.get("start_pos", 0))
    assert start_pos == 0, "kernel specialized for start_pos=0"

    from concourse.bass_utils import run_bass_kernel_spmd

    key = (DIM, S, TT)
    if key not in _NC_CACHE:
        _NC_CACHE[key] = build_module(DIM, S, TT)
    nc = _NC_CACHE[key]

    in_maps = _prep_in_maps(x, cos, sin, Wq, bq, Wk, bk, Wv, bv, Wo, mask,
                            DIM, S, TT)

    trace = bool(int(os.environ.get("KERNEL_TRACE", "0")))
    if trace:
        trace = _install_ntff_hook()
    res = run_bass_kernel_spmd(
        nc, in_maps, core_ids=list(range(NCORES)), trace=trace,
        trace_cores=list(range(NCORES)) if trace else None,
    )
    if trace:
        kernel.last_results = res

    fullT = np.concatenate([res.results[c]["y"] for c in range(NCORES)], axis=0)
    return np.ascontiguousarray(fullT.T).reshape(B, S, DIM).astype(np.float32)


# revision 34
# speedup vs baseline: 1.4387x; 1.4387x over previous
"""Trainium2 Bass kernel for nn_Attention_523986010726.

Dense GQA attention layer (B=2, S=2048, D=4096, 32 q-heads / 8 kv-heads,
head_dim=128, RoPE, causal mask, fused QKV+SDPA+output projection).

Sharding (per spec hint): tensor-parallel across heads over 8 NeuronCores.
Each core owns 1 kv-head + its 4 q-heads: Wq/Wk/Wv column-sharded,
Wo row-sharded.  Per-core partial outputs are combined with an on-chip
ReduceScatter (8 cores, chunked per 512-token tile so the collective
overlaps compute); each core ends up with a 512-row slice of the
(DIM x B*S) transposed output, which the host reassembles.

Device dataflow (everything "transposed": feature dims on SBUF partitions):
  xT[d,t] (host-pretransposed, bf16) --matmul--> Q^T/K^T/V^T per t-tile
  RoPE applied in [hd, t] layout (rotate-half via SBUF->SBUF DMA partition
  shift; 1/sqrt(hd) folded into K's cos/sin tables)
  S^T[k,t] = K^T.T @ Q^T per 128-k-chunk; causal mask added on diagonal
  band tiles; P^T = exp(S^T) on ScalarE (no max-subtraction: |scores|<~18)
  out^T[hd,t] += V_chunk.T @ P^T accumulated in PSUM; softmax denominators
  via a ones-vector matmul on the same P^T chunks; normalization delayed
  to after AV (everything is linear in the k-sum), applied as
  out^T * broadcast(1/den) where the broadcast across partitions is a
  rank-1 matmul.
  O-proj: partial^T[dim,t] += WoT_chunk.T @ attn^T, DMA'd to DRAM and
  ReduceScattered across the 8 cores.
"""

import os
import sys
from contextlib import ExitStack

sys.path.insert(0, "/opt/trn_rl_repo")

import numpy as np
import ml_dtypes

B, S, DIM = 2, 2048, 4096
HQ, HKV, HD = 32, 8, 128
NCORES = 8
EH = HQ // NCORES          # q-heads per core (4)
E = EH * HD                # per-core q-projection width (512)
TT = 512                   # token tile (matmul moving free dim)

BF16 = ml_dtypes.bfloat16


HOST_REDUCE = bool(int(os.environ.get("KERNEL_HOST_REDUCE", "0")))


def build_module(D=DIM, S_=S, TT_=TT, host_reduce=HOST_REDUCE):
    """Build the per-core Bass module (identical on all cores; per-core
    weight slices arrive as input values)."""
    import concourse.bass as bass  # noqa: F401
    import concourse.mybir as mybir
    import concourse.tile as tile
    from concourse import bacc

    f32 = mybir.dt.float32
    bf16 = mybir.dt.bfloat16
    AF = mybir.ActivationFunctionType

    DC = D // 128            # contraction chunks for projections
    NT_B = S_ // TT_         # t-tiles per batch
    NT = B * NT_B            # total t-tiles
    NPAT = TT_ // 128        # diagonal mask patterns
    KCB = S_ // 128          # k-chunks per batch
    DQ = 8 if DC % 8 == 0 else DC   # xT streaming sub-block (d-chunks)
    NDQ = DC // DQ

    nc = bacc.Bacc(num_devices=NCORES)

    xT = nc.dram_tensor("xT", [D, B * S_], bf16, kind="ExternalInput")
    wqT = nc.dram_tensor("wqT", [D, E], bf16, kind="ExternalInput")
    wkT = nc.dram_tensor("wkT", [D, HD], bf16, kind="ExternalInput")
    wvT = nc.dram_tensor("wvT", [D, HD], bf16, kind="ExternalInput")
    woT = nc.dram_tensor("woT", [E, D], bf16, kind="ExternalInput")
    bqc = nc.dram_tensor("bqc", [128, EH], f32, kind="ExternalInput")
    bkc = nc.dram_tensor("bkc", [128, 1], f32, kind="ExternalInput")
    bvc = nc.dram_tensor("bvc", [128, 1], f32, kind="ExternalInput")
    cosq = nc.dram_tensor("cosq", [128, S_], bf16, kind="ExternalInput")
    sinq = nc.dram_tensor("sinq", [128, S_], bf16, kind="ExternalInput")
    cosk = nc.dram_tensor("cosk", [128, S_], bf16, kind="ExternalInput")
    sink = nc.dram_tensor("sink", [128, S_], bf16, kind="ExternalInput")
    mpat = nc.dram_tensor("mpat", [128, NPAT, TT_], f32, kind="ExternalInput")
    idbf = nc.dram_tensor("idbf", [128, 128], bf16, kind="ExternalInput")
    if host_reduce:
        y = nc.dram_tensor("y", [D, B * S_], f32, kind="ExternalOutput")
        partials = rsouts = None
    else:
        y = nc.dram_tensor("y", [D // NCORES, B * S_], f32, kind="ExternalOutput")
        partials = [nc.dram_tensor(f"partial_{t}", [D, TT_], f32)
                    for t in range(NT)]
        rsouts = [nc.dram_tensor(f"rsout_{t}", [D // NCORES, TT_], f32)
                  for t in range(NT)]

    with tile.TileContext(nc) as tc, ExitStack() as ctx:
        const = ctx.enter_context(tc.tile_pool(name="const", bufs=1))
        xpool = ctx.enter_context(tc.tile_pool(name="xpool", bufs=NDQ + 1))
        rpool = ctx.enter_context(tc.tile_pool(name="rpool", bufs=3))
        qpool = ctx.enter_context(tc.tile_pool(name="qpool", bufs=2))
        ppool = ctx.enter_context(tc.tile_pool(name="ppool", bufs=4))
        apool = ctx.enter_context(tc.tile_pool(name="apool", bufs=2))
        opool = ctx.enter_context(tc.tile_pool(name="opool", bufs=3))
        spool = ctx.enter_context(tc.tile_pool(name="spool", bufs=2))

        # One shared PSUM pool: every phase can use all 8 banks, decoupling
        # the PE from the (slower) PSUM->SBUF drain engines.
        psum = ctx.enter_context(tc.tile_pool(name="psum", bufs=8, space="PSUM"))

        # ---- resident constants ----
        wq_sb = const.tile([128, DC, E], bf16)
        nc.sync.dma_start(wq_sb[:], wqT.ap().rearrange("(o p) e -> p o e", p=128))
        wk_sb = const.tile([128, DC, HD], bf16)
        nc.sync.dma_start(wk_sb[:], wkT.ap().rearrange("(o p) e -> p o e", p=128))
        wv_sb = const.tile([128, DC, HD], bf16)
        nc.sync.dma_start(wv_sb[:], wvT.ap().rearrange("(o p) e -> p o e", p=128))
        wo_sb = const.tile([128, EH, D], bf16)
        nc.sync.dma_start(wo_sb[:], woT.ap().rearrange("(o p) d -> p o d", p=128))
        bq_sb = const.tile([128, EH], f32)
        nc.sync.dma_start(bq_sb[:], bqc.ap())
        bk_sb = const.tile([128, 1], f32)
        nc.sync.dma_start(bk_sb[:], bkc.ap())
        bv_sb = const.tile([128, 1], f32)
        nc.sync.dma_start(bv_sb[:], bvc.ap())
        cosq_sb = const.tile([128, S_], bf16)
        nc.sync.dma_start(cosq_sb[:], cosq.ap())
        sinq_sb = const.tile([128, S_], bf16)
        nc.sync.dma_start(sinq_sb[:], sinq.ap())
        cosk_sb = const.tile([128, S_], bf16)
        nc.sync.dma_start(cosk_sb[:], cosk.ap())
        sink_sb = const.tile([128, S_], bf16)
        nc.sync.dma_start(sink_sb[:], sink.ap())
        mp_sb = const.tile([128, NPAT, TT_], f32)
        nc.sync.dma_start(mp_sb[:], mpat.ap())
        id_sb = const.tile([128, 128], bf16)
        nc.sync.dma_start(id_sb[:], idbf.ap())
        ones_col = const.tile([128, 1], bf16)
        nc.vector.memset(ones_col[:], 1.0)
        ones_row = const.tile([1, 128], f32)
        nc.vector.memset(ones_row[:], 1.0)

        # persistent K^T / V accumulation buffers (filled tile-by-tile)
        kT_sb = const.tile([128, B, S_], bf16)
        v_sb = const.tile([128, B, KCB, HD], bf16)

        xT_r = xT.ap().rearrange("(o p) t -> p o t", p=128)

        def rope(dst, src_f, cos_sb, sin_sb, t0):
            """dst[hd,t] = src*cos + rotate_half(src)*sin  (sin sign-folded).

            src_f: [128, TT] bf16 SBUF tile (pre-RoPE projection incl bias).
            """
            ssh = rpool.tile([128, TT_], bf16, tag="ssh")
            # partition rotate-by-64 via SBUF->SBUF DMA
            nc.sync.dma_start(ssh[0:64, :], src_f[64:128, :])
            nc.sync.dma_start(ssh[64:128, :], src_f[0:64, :])
            t1 = rpool.tile([128, TT_], bf16, tag="t1")
            nc.vector.tensor_mul(out=t1[:], in0=src_f[:], in1=cos_sb[:, t0:t0 + TT_])
            t2 = rpool.tile([128, TT_], bf16, tag="t2")
            nc.vector.tensor_mul(out=t2[:], in0=ssh[:], in1=sin_sb[:, t0:t0 + TT_])
            nc.vector.tensor_add(out=dst, in0=t1[:], in1=t2[:])

        for tt in range(NT):
            b = tt // NT_B
            t0 = (tt % NT_B) * TT_
            g0 = tt * TT_             # global token offset

            # ---- stream x^T block for this t-tile ----
            xq = []
            for qq in range(NDQ):
                xt_q = xpool.tile([128, DQ, TT_], bf16, tag="xt")
                nc.sync.dma_start(
                    xt_q[:], xT_r[:, qq * DQ:(qq + 1) * DQ, g0:g0 + TT_]
                )
                xq.append(xt_q)

            def proj_matmuls(ps, w_sb, esl):
                for dc in range(DC):
                    nc.tensor.matmul(
                        ps[:],
                        lhsT=w_sb[:, dc, esl],
                        rhs=xq[dc // DQ][:, dc % DQ, :],
                        start=(dc == 0),
                        stop=(dc == DC - 1),
                    )

            # ---- Q projection + RoPE ----
            q_blk = qpool.tile([128, EH, TT_], bf16)
            for e in range(EH):
                ps_q = psum.tile([128, TT_], f32, tag="ps")
                proj_matmuls(ps_q, wq_sb, slice(e * 128, (e + 1) * 128))
                qf = rpool.tile([128, TT_], bf16, tag="projf")
                nc.scalar.add(qf[:], ps_q[:], bq_sb[:, e:e + 1])
                rope(q_blk[:, e, :], qf, cosq_sb, sinq_sb, t0)

            # ---- K projection + RoPE (pre-scaled trig) ----
            ps_k = psum.tile([128, TT_], f32, tag="ps")
            proj_matmuls(ps_k, wk_sb, slice(0, HD))
            kf = rpool.tile([128, TT_], bf16, tag="projf")
            nc.scalar.add(kf[:], ps_k[:], bk_sb[:, 0:1])
            rope(kT_sb[:, b, t0:t0 + TT_], kf, cosk_sb, sink_sb, t0)

            # ---- V projection (V^T then transpose to natural [k, hd]) ----
            ps_v = psum.tile([128, TT_], f32, tag="ps")
            proj_matmuls(ps_v, wv_sb, slice(0, HD))
            vf = rpool.tile([128, TT_], bf16, tag="projf")
            nc.scalar.add(vf[:], ps_v[:], bv_sb[:, 0:1])
            for j in range(TT_ // 128):
                ps_t = psum.tile([128, 128], bf16, tag="ps")
                nc.tensor.transpose(ps_t[:], vf[:, j * 128:(j + 1) * 128], id_sb[:])
                nc.vector.tensor_copy(
                    out=v_sb[:, b, t0 // 128 + j, :], in_=ps_t[:]
                )

            # ---- attention for this query tile ----
            # Chunk order: interleave the 4 diagonal (masked) chunks between
            # full chunks so their extra DVE mask-add overlaps PE streaming.
            # The chunk loop is software-pipelined depth 2: AV/den matmuls for
            # chunk i issue after S^T/exp of chunk i+2, so the PE never waits
            # on ScalarE's exp.
            nkc = (t0 + TT_) // 128
            diag = list(range(t0 // 128, nkc))
            full = list(range(t0 // 128))
            order = []
            if full:
                stride = max(1, len(full) // len(diag))
                fi = 0
                for d_ in diag:
                    order.append(d_)
                    order.extend(full[fi:fi + stride])
                    fi += stride
                order.extend(full[fi:])
            else:
                order = diag
            assert sorted(order) == list(range(nkc))

            att = apool.tile([128, EH, TT_], bf16)
            pending = None  # (h, av, rec) epilogue deferred one head

            def epilogue(h_, av_, rec_):
                # broadcast 1/den across partitions (rank-1 matmul) and
                # normalize; rec_ was computed during the next head's chunk
                # loop so the bcast matmul never stalls the PE.
                bc = psum.tile([128, TT_], f32, tag="ps")
                nc.tensor.matmul(
                    bc[:], lhsT=ones_row[:], rhs=rec_[:], start=True, stop=True
                )
                avs = spool.tile([128, TT_], f32, tag="avs")
                nc.scalar.copy(avs[:], av_[:])
                nc.vector.tensor_mul(out=att[:, h_, :], in0=avs[:], in1=bc[:])

            for h in range(EH):
                av = psum.tile([128, TT_], f32, tag="ps")
                den = psum.tile([1, TT_], f32, tag="ps")
                DEPTH = 2
                pts = {}

                def av_den(i, h=h, av=av, den=den):
                    kc = order[i]
                    pt = pts.pop(i)
                    nc.tensor.matmul(
                        av[:], lhsT=v_sb[:, b, kc, :], rhs=pt[:],
                        start=(i == 0), stop=(i == nkc - 1),
                    )
                    nc.tensor.matmul(
                        den[:], lhsT=ones_col[:], rhs=pt[:],
                        start=(i == 0), stop=(i == nkc - 1),
                    )

                for i, kc in enumerate(order):
                    st = psum.tile([128, TT_], f32, tag="ps")
                    nc.tensor.matmul(
                        st[:],
                        lhsT=kT_sb[:, b, kc * 128:(kc + 1) * 128],
                        rhs=q_blk[:, h, :],
                        start=True,
                        stop=True,
                    )
                    d = kc * 128 - t0
                    if d >= 0:  # diagonal band: apply causal mask pattern
                        nc.vector.tensor_add(
                            out=st[:], in0=st[:], in1=mp_sb[:, d // 128, :]
                        )
                    pt = ppool.tile([128, TT_], bf16, tag="pt")
                    nc.scalar.activation(pt[:], st[:], AF.Exp)
                    pts[i] = pt
                    if i >= DEPTH:
                        av_den(i - DEPTH)
                for i in range(max(0, nkc - DEPTH), nkc):
                    av_den(i)

                rec = spool.tile([1, TT_], f32, tag="rec")
                nc.vector.reciprocal(rec[:], den[:])
                if pending is not None:
                    epilogue(*pending)
                pending = (h, av, rec)
            epilogue(*pending)

            # ---- output projection (partial, transposed) ----
            # PSUM->SBUF staging on DVE (keeps ScalarE free for next tile's
            # exps); partial writes on the ACT HWDGE queue so the SP queue
            # only carries latency-critical loads.
            for dt in range(DC):
                po = psum.tile([128, TT_], f32, tag="ps")
                for c in range(EH):
                    nc.tensor.matmul(
                        po[:],
                        lhsT=wo_sb[:, c, dt * 128:(dt + 1) * 128],
                        rhs=att[:, c, :],
                        start=(c == 0),
                        stop=(c == EH - 1),
                    )
                osb = opool.tile([128, TT_], f32, tag="osb")
                nc.scalar.copy(osb[:], po[:])
                if host_reduce:
                    nc.scalar.dma_start(
                        y.ap()[dt * 128:(dt + 1) * 128, g0:g0 + TT_], osb[:]
                    )
                else:
                    nc.scalar.dma_start(
                        partials[tt].ap()[dt * 128:(dt + 1) * 128, :], osb[:]
                    )

            if not host_reduce:
                # ---- chunked reduce-scatter + final copy ----
                import concourse.mybir as mybir_  # noqa: PLC0415
                nc.gpsimd.collective_compute(
                    "ReduceScatter",
                    mybir_.AluOpType.add,
                    replica_groups=[list(range(NCORES))],
                    ins=[partials[tt].ap().opt()],
                    outs=[rsouts[tt].ap().opt()],
                )
                nc.gpsimd.dma_start(y.ap()[:, g0:g0 + TT_], rsouts[tt].ap())

    nc.finalize()
    return nc


def _prep_in_maps(x, cos, sin, Wq, bq, Wk, bk, Wv, bv, Wo, mask, D, S_, TT_):
    """Host-side sharding/prep: transpose+cast per-core operand slices."""
    NPAT = TT_ // 128
    scaling = np.float32(1.0 / np.sqrt(HD))

    xT = np.ascontiguousarray(
        x.reshape(B * S_, D).T.astype(BF16)
    )  # [D, B*S]
    cosT = cos.T.astype(np.float32)            # [HD, S]
    sinT = sin.T.astype(np.float32)
    sgn = np.ones((HD, 1), np.float32)
    sgn[: HD // 2] = -1.0
    cosq_h = np.ascontiguousarray(cosT.astype(BF16))
    sinq_h = np.ascontiguousarray((sinT * sgn).astype(BF16))
    cosk_h = np.ascontiguousarray((cosT * scaling).astype(BF16))
    sink_h = np.ascontiguousarray((sinT * sgn * scaling).astype(BF16))

    # causal mask diagonal patterns from the mask input:
    # mpat[p, i, f] = clip(mask[f, i*128 + p], -100, 0)
    m = np.asarray(mask).reshape(np.asarray(mask).shape[-2], -1)[:TT_, :TT_]
    mp = np.clip(m.T, -100.0, 0.0).astype(np.float32)        # [k, t]
    mpat = np.ascontiguousarray(
        mp.reshape(NPAT, 128, TT_).transpose(1, 0, 2)
    )  # [p, i, f]

    idbf = np.eye(128, dtype=BF16)

    in_maps = []
    for c in range(NCORES):
        e0 = c * E
        k0 = c * HD
        in_maps.append({
            "xT": xT,
            "wqT": np.ascontiguousarray(Wq[e0:e0 + E].T.astype(BF16)),
            "wkT": np.ascontiguousarray(Wk[k0:k0 + HD].T.astype(BF16)),
            "wvT": np.ascontiguousarray(Wv[k0:k0 + HD].T.astype(BF16)),
            "woT": np.ascontiguousarray(Wo[:, e0:e0 + E].T.astype(BF16)),
            "bqc": np.ascontiguousarray(
                bq[e0:e0 + E].reshape(EH, 128).T.astype(np.float32)
            ),
            "bkc": bk[k0:k0 + HD].reshape(HD, 1).astype(np.float32),
            "bvc": bv[k0:k0 + HD].reshape(HD, 1).astype(np.float32),
            "cosq": cosq_h,
            "sinq": sinq_h,
            "cosk": cosk_h,
            "sink": sink_h,
            "mpat": mpat,
            "idbf": idbf,
        })
    return in_maps


_NC_CACHE = {}


def _install_ntff_hook():
    """Register the axon NTFF profiling hook that this image's antenv lacks
    (used only when KERNEL_TRACE=1; grading runs without it)."""
    import types

    try:
        import antenv
        if "antenv.axon_hooks" not in sys.modules:
            mod = types.ModuleType("antenv.axon_hooks")
            state = {"hook": None}
            mod.set_axon_ntff_profile_hook = lambda h: state.__setitem__("hook", h)
            mod.get_axon_ntff_profile_hook = lambda: state["hook"]
            sys.modules["antenv.axon_hooks"] = mod
            antenv.axon_hooks = mod
        import antenv.axon_hooks as ah
        if ah.get_axon_ntff_profile_hook() is None:
            from trn_agent_boot.trn_boot import _ntff_profile_via_ctypes
            ah.set_axon_ntff_profile_hook(
                _ntff_profile_via_ctypes("/opt/axon/libaxon_pjrt.so")
            )
        return True
    except Exception as e:  # pragma: no cover - best effort
        print(f"NTFF hook install failed ({e}); running without trace")
        return False


def kernel(**inputs):
    x = np.asarray(inputs["x"], np.float32)
    cos = np.asarray(inputs["cos"], np.float32)
    sin = np.asarray(inputs["sin"], np.float32)
    Wq = np.asarray(inputs["Wq"], np.float32)
    bq = np.asarray(inputs["bq"], np.float32)
    Wk = np.asarray(inputs["Wk"], np.float32)
    bk = np.asarray(inputs["bk"], np.float32)
    Wv = np.asarray(inputs["Wv"], np.float32)
    bv = np.asarray(inputs["bv"], np.float32)
    Wo = np.asarray(inputs["Wo"], np.float32)
    mask = np.asarray(inputs["mask"], np.float32)
    start_pos = int(inputs.get("start_pos", 0))
    assert start_pos == 0, "kernel specialized for start_pos=0"

    from concourse.bass_utils import run_bass_kernel_spmd

    key = (DIM, S, TT)
    if key not in _NC_CACHE:
        _NC_CACHE[key] = build_module(DIM, S, TT)
    nc = _NC_CACHE[key]

    in_maps = _prep_in_maps(x, cos, sin, Wq, bq, Wk, bk, Wv, bv, Wo, mask,
                            DIM, S, TT)

    trace = bool(int(os.environ.get("KERNEL_TRACE", "0")))
    if trace:
        trace = _install_ntff_hook()
    res = run_bass_kernel_spmd(
        nc, in_maps, core_ids=list(range(NCORES)), trace=trace,
        trace_cores=list(range(NCORES)) if trace else None,
    )
    if trace:
        kernel.last_results = res

    if HOST_REDUCE:
        fullT = res.results[0]["y"].astype(np.float64)
        for c in range(1, NCORES):
            fullT += res.results[c]["y"]
        fullT = fullT.astype(np.float32)
    else:
        fullT = np.concatenate([res.results[c]["y"] for c in range(NCORES)],
                               axis=0)
    return np.ascontiguousarray(fullT.T).reshape(B, S, DIM).astype(np.float32)


# revision 36
# speedup vs baseline: 1.4430x; 1.0030x over previous
"""Trainium2 Bass kernel for nn_Attention_523986010726.

Dense GQA attention layer (B=2, S=2048, D=4096, 32 q-heads / 8 kv-heads,
head_dim=128, RoPE, causal mask, fused QKV+SDPA+output projection).

Sharding (per spec hint): tensor-parallel across heads over 8 NeuronCores.
Each core owns 1 kv-head + its 4 q-heads: Wq/Wk/Wv column-sharded,
Wo row-sharded.  Per-core partial outputs are combined with an on-chip
ReduceScatter (8 cores, chunked per 512-token tile so the collective
overlaps compute); each core ends up with a 512-row slice of the
(DIM x B*S) transposed output, which the host reassembles.

Device dataflow (everything "transposed": feature dims on SBUF partitions):
  xT[d,t] (host-pretransposed, bf16) --matmul--> Q^T/K^T/V^T per t-tile
  RoPE applied in [hd, t] layout (rotate-half via SBUF->SBUF DMA partition
  shift; 1/sqrt(hd) folded into K's cos/sin tables)
  S^T[k,t] = K^T.T @ Q^T per 128-k-chunk; causal mask added on diagonal
  band tiles; P^T = exp(S^T) on ScalarE (no max-subtraction: |scores|<~18)
  out^T[hd,t] += V_chunk.T @ P^T accumulated in PSUM; softmax denominators
  via a ones-vector matmul on the same P^T chunks; normalization delayed
  to after AV (everything is linear in the k-sum), applied as
  out^T * broadcast(1/den) where the broadcast across partitions is a
  rank-1 matmul.
  O-proj: partial^T[dim,t] += WoT_chunk.T @ attn^T, DMA'd to DRAM and
  ReduceScattered across the 8 cores.
"""

import os
import sys
from contextlib import ExitStack

sys.path.insert(0, "/opt/trn_rl_repo")

import numpy as np
import ml_dtypes

B, S, DIM = 2, 2048, 4096
HQ, HKV, HD = 32, 8, 128
NCORES = 8
EH = HQ // NCORES          # q-heads per core (4)
E = EH * HD                # per-core q-projection width (512)
TT = 512                   # token tile (matmul moving free dim)

BF16 = ml_dtypes.bfloat16


HOST_REDUCE = bool(int(os.environ.get("KERNEL_HOST_REDUCE", "0")))


def build_module(D=DIM, S_=S, TT_=TT, host_reduce=HOST_REDUCE):
    """Build the per-core Bass module (identical on all cores; per-core
    weight slices arrive as input values)."""
    import concourse.bass as bass  # noqa: F401
    import concourse.mybir as mybir
    import concourse.tile as tile
    from concourse import bacc

    f32 = mybir.dt.float32
    bf16 = mybir.dt.bfloat16
    AF = mybir.ActivationFunctionType

    DC = D // 128            # contraction chunks for projections
    NT_B = S_ // TT_         # t-tiles per batch
    NT = B * NT_B            # total t-tiles
    NPAT = TT_ // 128        # diagonal mask patterns
    KCB = S_ // 128          # k-chunks per batch
    DQ = 8 if DC % 8 == 0 else DC   # xT streaming sub-block (d-chunks)
    NDQ = DC // DQ

    nc = bacc.Bacc(num_devices=NCORES)

    xT = nc.dram_tensor("xT", [D, B * S_], bf16, kind="ExternalInput")
    wqT = nc.dram_tensor("wqT", [D, E], bf16, kind="ExternalInput")
    wkT = nc.dram_tensor("wkT", [D, HD], bf16, kind="ExternalInput")
    wvT = nc.dram_tensor("wvT", [D, HD], bf16, kind="ExternalInput")
    woT = nc.dram_tensor("woT", [E, D], bf16, kind="ExternalInput")
    bqc = nc.dram_tensor("bqc", [128, EH], f32, kind="ExternalInput")
    bkc = nc.dram_tensor("bkc", [128, 1], f32, kind="ExternalInput")
    bvc = nc.dram_tensor("bvc", [128, 1], f32, kind="ExternalInput")
    cosq = nc.dram_tensor("cosq", [128, S_], bf16, kind="ExternalInput")
    sinq = nc.dram_tensor("sinq", [128, S_], bf16, kind="ExternalInput")
    cosk = nc.dram_tensor("cosk", [128, S_], bf16, kind="ExternalInput")
    sink = nc.dram_tensor("sink", [128, S_], bf16, kind="ExternalInput")
    mpat = nc.dram_tensor("mpat", [128, NPAT, TT_], f32, kind="ExternalInput")
    idbf = nc.dram_tensor("idbf", [128, 128], bf16, kind="ExternalInput")
    if host_reduce:
        y = nc.dram_tensor("y", [D, B * S_], f32, kind="ExternalOutput")
        partials = rsouts = None
    else:
        y = nc.dram_tensor("y", [D // NCORES, B * S_], f32, kind="ExternalOutput")
        partials = [nc.dram_tensor(f"partial_{t}", [D, TT_], f32)
                    for t in range(NT)]
        rsouts = [nc.dram_tensor(f"rsout_{t}", [D // NCORES, TT_], f32)
                  for t in range(NT)]

    with tile.TileContext(nc) as tc, ExitStack() as ctx:
        const = ctx.enter_context(tc.tile_pool(name="const", bufs=1))
        xpool = ctx.enter_context(tc.tile_pool(name="xpool", bufs=NDQ + 1))
        rpool = ctx.enter_context(tc.tile_pool(name="rpool", bufs=3))
        qpool = ctx.enter_context(tc.tile_pool(name="qpool", bufs=2))
        ppool = ctx.enter_context(tc.tile_pool(name="ppool", bufs=4))
        apool = ctx.enter_context(tc.tile_pool(name="apool", bufs=2))
        opool = ctx.enter_context(tc.tile_pool(name="opool", bufs=3))
        spool = ctx.enter_context(tc.tile_pool(name="spool", bufs=2))

        # One shared PSUM pool: every phase can use all 8 banks, decoupling
        # the PE from the (slower) PSUM->SBUF drain engines.
        psum = ctx.enter_context(tc.tile_pool(name="psum", bufs=8, space="PSUM"))

        # ---- resident constants ----
        wq_sb = const.tile([128, DC, E], bf16)
        nc.sync.dma_start(wq_sb[:], wqT.ap().rearrange("(o p) e -> p o e", p=128))
        wk_sb = const.tile([128, DC, HD], bf16)
        nc.sync.dma_start(wk_sb[:], wkT.ap().rearrange("(o p) e -> p o e", p=128))
        wv_sb = const.tile([128, DC, HD], bf16)
        nc.sync.dma_start(wv_sb[:], wvT.ap().rearrange("(o p) e -> p o e", p=128))
        wo_sb = const.tile([128, EH, D], bf16)
        nc.sync.dma_start(wo_sb[:], woT.ap().rearrange("(o p) d -> p o d", p=128))
        bq_sb = const.tile([128, EH], f32)
        nc.sync.dma_start(bq_sb[:], bqc.ap())
        bk_sb = const.tile([128, 1], f32)
        nc.sync.dma_start(bk_sb[:], bkc.ap())
        bv_sb = const.tile([128, 1], f32)
        nc.sync.dma_start(bv_sb[:], bvc.ap())
        cosq_sb = const.tile([128, S_], bf16)
        nc.sync.dma_start(cosq_sb[:], cosq.ap())
        sinq_sb = const.tile([128, S_], bf16)
        nc.sync.dma_start(sinq_sb[:], sinq.ap())
        cosk_sb = const.tile([128, S_], bf16)
        nc.sync.dma_start(cosk_sb[:], cosk.ap())
        sink_sb = const.tile([128, S_], bf16)
        nc.sync.dma_start(sink_sb[:], sink.ap())
        mp_sb = const.tile([128, NPAT, TT_], f32)
        nc.sync.dma_start(mp_sb[:], mpat.ap())
        id_sb = const.tile([128, 128], bf16)
        nc.sync.dma_start(id_sb[:], idbf.ap())
        ones_col = const.tile([128, 1], bf16)
        nc.vector.memset(ones_col[:], 1.0)
        ones_row = const.tile([1, 128], f32)
        nc.vector.memset(ones_row[:], 1.0)

        # persistent K^T / V accumulation buffers (filled tile-by-tile)
        kT_sb = const.tile([128, B, S_], bf16)
        v_sb = const.tile([128, B, KCB, HD], bf16)

        xT_r = xT.ap().rearrange("(o p) t -> p o t", p=128)

        def rope(dst, src_f, cos_sb, sin_sb, t0):
            """dst[hd,t] = src*cos + rotate_half(src)*sin  (sin sign-folded).

            src_f: [128, TT] bf16 SBUF tile (pre-RoPE projection incl bias).
            """
            ssh = rpool.tile([128, TT_], bf16, tag="ssh")
            # partition rotate-by-64 via SBUF->SBUF DMA
            nc.sync.dma_start(ssh[0:64, :], src_f[64:128, :])
            nc.sync.dma_start(ssh[64:128, :], src_f[0:64, :])
            t1 = rpool.tile([128, TT_], bf16, tag="t1")
            nc.vector.tensor_mul(out=t1[:], in0=src_f[:], in1=cos_sb[:, t0:t0 + TT_])
            t2 = rpool.tile([128, TT_], bf16, tag="t2")
            nc.vector.tensor_mul(out=t2[:], in0=ssh[:], in1=sin_sb[:, t0:t0 + TT_])
            nc.vector.tensor_add(out=dst, in0=t1[:], in1=t2[:])

        for tt in range(NT):
            b = tt // NT_B
            t0 = (tt % NT_B) * TT_
            g0 = tt * TT_             # global token offset

            # ---- stream x^T block for this t-tile ----
            xq = []
            for qq in range(NDQ):
                xt_q = xpool.tile([128, DQ, TT_], bf16, tag="xt")
                nc.sync.dma_start(
                    xt_q[:], xT_r[:, qq * DQ:(qq + 1) * DQ, g0:g0 + TT_]
                )
                xq.append(xt_q)

            def proj_matmuls(ps, w_sb, esl):
                for dc in range(DC):
                    nc.tensor.matmul(
                        ps[:],
                        lhsT=w_sb[:, dc, esl],
                        rhs=xq[dc // DQ][:, dc % DQ, :],
                        start=(dc == 0),
                        stop=(dc == DC - 1),
                    )

            # ---- Q projection + RoPE ----
            q_blk = qpool.tile([128, EH, TT_], bf16)
            for e in range(EH):
                ps_q = psum.tile([128, TT_], f32, tag="ps")
                proj_matmuls(ps_q, wq_sb, slice(e * 128, (e + 1) * 128))
                qf = rpool.tile([128, TT_], bf16, tag="projf")
                nc.scalar.add(qf[:], ps_q[:], bq_sb[:, e:e + 1])
                rope(q_blk[:, e, :], qf, cosq_sb, sinq_sb, t0)

            # ---- K projection + RoPE (pre-scaled trig) ----
            ps_k = psum.tile([128, TT_], f32, tag="ps")
            proj_matmuls(ps_k, wk_sb, slice(0, HD))
            kf = rpool.tile([128, TT_], bf16, tag="projf")
            nc.scalar.add(kf[:], ps_k[:], bk_sb[:, 0:1])
            rope(kT_sb[:, b, t0:t0 + TT_], kf, cosk_sb, sink_sb, t0)

            # ---- V projection (V^T then transpose to natural [k, hd]) ----
            ps_v = psum.tile([128, TT_], f32, tag="ps")
            proj_matmuls(ps_v, wv_sb, slice(0, HD))
            vf = rpool.tile([128, TT_], bf16, tag="projf")
            nc.scalar.add(vf[:], ps_v[:], bv_sb[:, 0:1])
            for j in range(TT_ // 128):
                ps_t = psum.tile([128, 128], bf16, tag="ps")
                nc.tensor.transpose(ps_t[:], vf[:, j * 128:(j + 1) * 128], id_sb[:])
                nc.vector.tensor_copy(
                    out=v_sb[:, b, t0 // 128 + j, :], in_=ps_t[:]
                )

            # ---- attention for this query tile ----
            # Chunk order: interleave the 4 diagonal (masked) chunks between
            # full chunks so their extra DVE mask-add overlaps PE streaming.
            # The chunk loop is software-pipelined depth 2: AV/den matmuls for
            # chunk i issue after S^T/exp of chunk i+2, so the PE never waits
            # on ScalarE's exp.
            nkc = (t0 + TT_) // 128
            diag = list(range(t0 // 128, nkc))
            full = list(range(t0 // 128))
            order = []
            if full:
                stride = max(1, len(full) // len(diag))
                fi = 0
                for d_ in diag:
                    order.append(d_)
                    order.extend(full[fi:fi + stride])
                    fi += stride
                order.extend(full[fi:])
            else:
                order = diag
            assert sorted(order) == list(range(nkc))

            att = apool.tile([128, EH, TT_], bf16)
            pending = None  # (h, av, rec) epilogue deferred one head

            def epilogue(h_, av_, rec_):
                # broadcast 1/den across partitions (rank-1 matmul) and
                # normalize; rec_ was computed during the next head's chunk
                # loop so the bcast matmul never stalls the PE.
                bc = psum.tile([128, TT_], f32, tag="ps")
                nc.tensor.matmul(
                    bc[:], lhsT=ones_row[:], rhs=rec_[:], start=True, stop=True
                )
                avs = spool.tile([128, TT_], f32, tag="avs")
                nc.vector.tensor_copy(out=avs[:], in_=av_[:])
                nc.vector.tensor_mul(out=att[:, h_, :], in0=avs[:], in1=bc[:])

            for h in range(EH):
                av = psum.tile([128, TT_], f32, tag="ps")
                den = psum.tile([1, TT_], f32, tag="ps")
                DEPTH = 2
                pts = {}

                def av_den(i, h=h, av=av, den=den):
                    kc = order[i]
                    pt = pts.pop(i)
                    nc.tensor.matmul(
                        av[:], lhsT=v_sb[:, b, kc, :], rhs=pt[:],
                        start=(i == 0), stop=(i == nkc - 1),
                    )
                    nc.tensor.matmul(
                        den[:], lhsT=ones_col[:], rhs=pt[:],
                        start=(i == 0), stop=(i == nkc - 1),
                    )

                for i, kc in enumerate(order):
                    st = psum.tile([128, TT_], f32, tag="ps")
                    nc.tensor.matmul(
                        st[:],
                        lhsT=kT_sb[:, b, kc * 128:(kc + 1) * 128],
                        rhs=q_blk[:, h, :],
                        start=True,
                        stop=True,
                    )
                    d = kc * 128 - t0
                    if d >= 0:  # diagonal band: apply causal mask pattern
                        nc.vector.tensor_add(
                            out=st[:], in0=st[:], in1=mp_sb[:, d // 128, :]
                        )
                    pt = ppool.tile([128, TT_], bf16, tag="pt")
                    nc.scalar.activation(pt[:], st[:], AF.Exp)
                    pts[i] = pt
                    if i >= DEPTH:
                        av_den(i - DEPTH)
                for i in range(max(0, nkc - DEPTH), nkc):
                    av_den(i)

                rec = spool.tile([1, TT_], f32, tag="rec")
                nc.vector.reciprocal(rec[:], den[:])
                if pending is not None:
                    epilogue(*pending)
                pending = (h, av, rec)
            epilogue(*pending)

            # ---- output projection (partial, transposed) ----
            # PSUM->SBUF staging on DVE (keeps ScalarE free for next tile's
            # exps); partial writes on the ACT HWDGE queue so the SP queue
            # only carries latency-critical loads.
            for dt in range(DC):
                po = psum.tile([128, TT_], f32, tag="ps")
                for c in range(EH):
                    nc.tensor.matmul(
                        po[:],
                        lhsT=wo_sb[:, c, dt * 128:(dt + 1) * 128],
                        rhs=att[:, c, :],
                        start=(c == 0),
                        stop=(c == EH - 1),
                    )
                osb = opool.tile([128, TT_], f32, tag="osb")
                if dt % 2 == 0:
                    nc.scalar.copy(osb[:], po[:])
                else:
                    nc.vector.tensor_copy(out=osb[:], in_=po[:])
                if host_reduce:
                    nc.scalar.dma_start(
                        y.ap()[dt * 128:(dt + 1) * 128, g0:g0 + TT_], osb[:]
                    )
                else:
                    nc.scalar.dma_start(
                        partials[tt].ap()[dt * 128:(dt + 1) * 128, :], osb[:]
                    )

            if not host_reduce:
                # ---- chunked reduce-scatter + final copy ----
                import concourse.mybir as mybir_  # noqa: PLC0415
                nc.gpsimd.collective_compute(
                    "ReduceScatter",
                    mybir_.AluOpType.add,
                    replica_groups=[list(range(NCORES))],
                    ins=[partials[tt].ap().opt()],
                    outs=[rsouts[tt].ap().opt()],
                )
                nc.gpsimd.dma_start(y.ap()[:, g0:g0 + TT_], rsouts[tt].ap())

    nc.finalize()
    return nc


def _prep_in_maps(x, cos, sin, Wq, bq, Wk, bk, Wv, bv, Wo, mask, D, S_, TT_):
    """Host-side sharding/prep: transpose+cast per-core operand slices."""
    NPAT = TT_ // 128
    scaling = np.float32(1.0 / np.sqrt(HD))

    xT = np.ascontiguousarray(
        x.reshape(B * S_, D).T.astype(BF16)
    )  # [D, B*S]
    cosT = cos.T.astype(np.float32)            # [HD, S]
    sinT = sin.T.astype(np.float32)
    sgn = np.ones((HD, 1), np.float32)
    sgn[: HD // 2] = -1.0
    cosq_h = np.ascontiguousarray(cosT.astype(BF16))
    sinq_h = np.ascontiguousarray((sinT * sgn).astype(BF16))
    cosk_h = np.ascontiguousarray((cosT * scaling).astype(BF16))
    sink_h = np.ascontiguousarray((sinT * sgn * scaling).astype(BF16))

    # causal mask diagonal patterns from the mask input:
    # mpat[p, i, f] = clip(mask[f, i*128 + p], -100, 0)
    m = np.asarray(mask).reshape(np.asarray(mask).shape[-2], -1)[:TT_, :TT_]
    mp = np.clip(m.T, -100.0, 0.0).astype(np.float32)        # [k, t]
    mpat = np.ascontiguousarray(
        mp.reshape(NPAT, 128, TT_).transpose(1, 0, 2)
    )  # [p, i, f]

    idbf = np.eye(128, dtype=BF16)

    in_maps = []
    for c in range(NCORES):
        e0 = c * E
        k0 = c * HD
        in_maps.append({
            "xT": xT,
            "wqT": np.ascontiguousarray(Wq[e0:e0 + E].T.astype(BF16)),
            "wkT": np.ascontiguousarray(Wk[k0:k0 + HD].T.astype(BF16)),
            "wvT": np.ascontiguousarray(Wv[k0:k0 + HD].T.astype(BF16)),
            "woT": np.ascontiguousarray(Wo[:, e0:e0 + E].T.astype(BF16)),
            "bqc": np.ascontiguousarray(
                bq[e0:e0 + E].reshape(EH, 128).T.astype(np.float32)
            ),
            "bkc": bk[k0:k0 + HD].reshape(HD, 1).astype(np.float32),
            "bvc": bv[k0:k0 + HD].reshape(HD, 1).astype(np.float32),
            "cosq": cosq_h,
            "sinq": sinq_h,
            "cosk": cosk_h,
            "sink": sink_h,
            "mpat": mpat,
            "idbf": idbf,
        })
    return in_maps


_NC_CACHE = {}


def _install_ntff_hook():
    """Register the axon NTFF profiling hook that this image's antenv lacks
    (used only when KERNEL_TRACE=1; grading runs without it)."""
    import types

    try:
        import antenv
        if "antenv.axon_hooks" not in sys.modules:
            mod = types.ModuleType("antenv.axon_hooks")
            state = {"hook": None}
            mod.set_axon_ntff_profile_hook = lambda h: state.__setitem__("hook", h)
            mod.get_axon_ntff_profile_hook = lambda: state["hook"]
            sys.modules["antenv.axon_hooks"] = mod
            antenv.axon_hooks = mod
        import antenv.axon_hooks as ah
        if ah.get_axon_ntff_profile_hook() is None:
            from trn_agent_boot.trn_boot import _ntff_profile_via_ctypes
            ah.set_axon_ntff_profile_hook(
                _ntff_profile_via_ctypes("/opt/axon/libaxon_pjrt.so")
            )
        return True
    except Exception as e:  # pragma: no cover - best effort
        print(f"NTFF hook install failed ({e}); running without trace")
        return False


def kernel(**inputs):
    x = np.asarray(inputs["x"], np.float32)
    cos = np.asarray(inputs["cos"], np.float32)
    sin = np.asarray(inputs["sin"], np.float32)
    Wq = np.asarray(inputs["Wq"], np.float32)
    bq = np.asarray(inputs["bq"], np.float32)
    Wk = np.asarray(inputs["Wk"], np.float32)
    bk = np.asarray(inputs["bk"], np.float32)
    Wv = np.asarray(inputs["Wv"], np.float32)
    bv = np.asarray(inputs["bv"], np.float32)
    Wo = np.asarray(inputs["Wo"], np.float32)
    mask = np.asarray(inputs["mask"], np.float32)
    start_pos = int(inputs.get("start_pos", 0))
    assert start_pos == 0, "kernel specialized for start_pos=0"

    from concourse.bass_utils import run_bass_kernel_spmd

    key = (DIM, S, TT)
    if key not in _NC_CACHE:
        _NC_CACHE[key] = build_module(DIM, S, TT)
    nc = _NC_CACHE[key]

    in_maps = _prep_in_maps(x, cos, sin, Wq, bq, Wk, bk, Wv, bv, Wo, mask,
                            DIM, S, TT)

    trace = bool(int(os.environ.get("KERNEL_TRACE", "0")))
    if trace:
        trace = _install_ntff_hook()
    res = run_bass_kernel_spmd(
        nc, in_maps, core_ids=list(range(NCORES)), trace=trace,
        trace_cores=list(range(NCORES)) if trace else None,
    )
    if trace:
        kernel.last_results = res

    if HOST_REDUCE:
        fullT = res.results[0]["y"].astype(np.float64)
        for c in range(1, NCORES):
            fullT += res.results[c]["y"]
        fullT = fullT.astype(np.float32)
    else:
        fullT = np.concatenate([res.results[c]["y"] for c in range(NCORES)],
                               axis=0)
    return np.ascontiguousarray(fullT.T).reshape(B, S, DIM).astype(np.float32)


# revision 44
# speedup vs baseline: 1.4560x; 1.0090x over previous
"""Trainium2 Bass kernel for nn_Attention_523986010726.

Dense GQA attention layer (B=2, S=2048, D=4096, 32 q-heads / 8 kv-heads,
head_dim=128, RoPE, causal mask, fused QKV+SDPA+output projection).

Sharding (per spec hint): tensor-parallel across heads over 8 NeuronCores.
Each core owns 1 kv-head + its 4 q-heads: Wq/Wk/Wv column-sharded,
Wo row-sharded.  Per-core partial outputs are combined with an on-chip
ReduceScatter (8 cores, chunked per 512-token tile so the collective
overlaps compute); each core ends up with a 512-row slice of the
(DIM x B*S) transposed output, which the host reassembles.

Device dataflow (everything "transposed": feature dims on SBUF partitions):
  xT[d,t] (host-pretransposed, bf16) --matmul--> Q^T/K^T/V^T per t-tile
  RoPE applied in [hd, t] layout (rotate-half via SBUF->SBUF DMA partition
  shift; 1/sqrt(hd) folded into K's cos/sin tables)
  S^T[k,t] = K^T.T @ Q^T per 128-k-chunk; causal mask added on diagonal
  band tiles; P^T = exp(S^T) on ScalarE (no max-subtraction: |scores|<~18)
  out^T[hd,t] += V_chunk.T @ P^T accumulated in PSUM; softmax denominators
  via a ones-vector matmul on the same P^T chunks; normalization delayed
  to after AV (everything is linear in the k-sum), applied as
  out^T * broadcast(1/den) where the broadcast across partitions is a
  rank-1 matmul.
  O-proj: partial^T[dim,t] += WoT_chunk.T @ attn^T, DMA'd to DRAM and
  ReduceScattered across the 8 cores.
"""

import os
import sys
from contextlib import ExitStack

sys.path.insert(0, "/opt/trn_rl_repo")

import numpy as np
import ml_dtypes

B, S, DIM = 2, 2048, 4096
HQ, HKV, HD = 32, 8, 128
NCORES = 8
EH = HQ // NCORES          # q-heads per core (4)
E = EH * HD                # per-core q-projection width (512)
TT = 512                   # token tile (matmul moving free dim)

BF16 = ml_dtypes.bfloat16


HOST_REDUCE = bool(int(os.environ.get("KERNEL_HOST_REDUCE", "0")))


def build_module(D=DIM, S_=S, TT_=TT, host_reduce=HOST_REDUCE):
    """Build the per-core Bass module (identical on all cores; per-core
    weight slices arrive as input values)."""
    import concourse.bass as bass  # noqa: F401
    import concourse.mybir as mybir
    import concourse.tile as tile
    from concourse import bacc

    f32 = mybir.dt.float32
    bf16 = mybir.dt.bfloat16
    AF = mybir.ActivationFunctionType

    DC = D // 128            # contraction chunks for projections
    NT_B = S_ // TT_         # t-tiles per batch
    NT = B * NT_B            # total t-tiles
    NPAT = TT_ // 128        # diagonal mask patterns
    KCB = S_ // 128          # k-chunks per batch
    DQ = 8 if DC % 8 == 0 else DC   # xT streaming sub-block (d-chunks)
    NDQ = DC // DQ

    nc = bacc.Bacc(num_devices=NCORES)

    xT = nc.dram_tensor("xT", [D, B * S_], bf16, kind="ExternalInput")
    wqT = nc.dram_tensor("wqT", [D, E], bf16, kind="ExternalInput")
    wkT = nc.dram_tensor("wkT", [D, HD], bf16, kind="ExternalInput")
    wvT = nc.dram_tensor("wvT", [D, HD], bf16, kind="ExternalInput")
    woT = nc.dram_tensor("woT", [E, D], bf16, kind="ExternalInput")
    bqc = nc.dram_tensor("bqc", [128, EH], f32, kind="ExternalInput")
    bkc = nc.dram_tensor("bkc", [128, 1], f32, kind="ExternalInput")
    bvc = nc.dram_tensor("bvc", [128, 1], f32, kind="ExternalInput")
    cosq = nc.dram_tensor("cosq", [128, S_], bf16, kind="ExternalInput")
    sinq = nc.dram_tensor("sinq", [128, S_], bf16, kind="ExternalInput")
    cosk = nc.dram_tensor("cosk", [128, S_], bf16, kind="ExternalInput")
    sink = nc.dram_tensor("sink", [128, S_], bf16, kind="ExternalInput")
    mpat = nc.dram_tensor("mpat", [128, NPAT, TT_], f32, kind="ExternalInput")
    idbf = nc.dram_tensor("idbf", [128, 128], bf16, kind="ExternalInput")
    if host_reduce:
        y = nc.dram_tensor("y", [D, B * S_], f32, kind="ExternalOutput")
        partials = rsouts = None
    else:
        y = nc.dram_tensor("y", [D // NCORES, B * S_], f32, kind="ExternalOutput")
        partials = [nc.dram_tensor(f"partial_{t}", [D, TT_], f32)
                    for t in range(NT)]
        rsouts = [nc.dram_tensor(f"rsout_{t}", [D // NCORES, TT_], f32)
                  for t in range(NT)]

    with tile.TileContext(nc) as tc, ExitStack() as ctx:
        const = ctx.enter_context(tc.tile_pool(name="const", bufs=1))
        xpool = ctx.enter_context(tc.tile_pool(name="xpool", bufs=NDQ + 1))
        rpool = ctx.enter_context(tc.tile_pool(name="rpool", bufs=3))
        qpool = ctx.enter_context(tc.tile_pool(name="qpool", bufs=2))
        ppool = ctx.enter_context(tc.tile_pool(name="ppool", bufs=6))
        apool = ctx.enter_context(tc.tile_pool(name="apool", bufs=2))
        opool = ctx.enter_context(tc.tile_pool(name="opool", bufs=3))
        spool = ctx.enter_context(tc.tile_pool(name="spool", bufs=2))

        # One shared PSUM pool: every phase can use all 8 banks, decoupling
        # the PE from the (slower) PSUM->SBUF drain engines.
        psum = ctx.enter_context(tc.tile_pool(name="psum", bufs=8, space="PSUM"))

        # ---- resident constants ----
        wq_sb = const.tile([128, DC, E], bf16)
        nc.sync.dma_start(wq_sb[:], wqT.ap().rearrange("(o p) e -> p o e", p=128))
        wk_sb = const.tile([128, DC, HD], bf16)
        nc.sync.dma_start(wk_sb[:], wkT.ap().rearrange("(o p) e -> p o e", p=128))
        wv_sb = const.tile([128, DC, HD], bf16)
        nc.sync.dma_start(wv_sb[:], wvT.ap().rearrange("(o p) e -> p o e", p=128))
        wo_sb = const.tile([128, EH, D], bf16)
        nc.sync.dma_start(wo_sb[:], woT.ap().rearrange("(o p) d -> p o d", p=128))
        bq_sb = const.tile([128, EH], f32)
        nc.sync.dma_start(bq_sb[:], bqc.ap())
        bk_sb = const.tile([128, 1], f32)
        nc.sync.dma_start(bk_sb[:], bkc.ap())
        bv_sb = const.tile([128, 1], f32)
        nc.sync.dma_start(bv_sb[:], bvc.ap())
        cosq_sb = const.tile([128, S_], bf16)
        nc.sync.dma_start(cosq_sb[:], cosq.ap())
        sinq_sb = const.tile([128, S_], bf16)
        nc.sync.dma_start(sinq_sb[:], sinq.ap())
        cosk_sb = const.tile([128, S_], bf16)
        nc.sync.dma_start(cosk_sb[:], cosk.ap())
        sink_sb = const.tile([128, S_], bf16)
        nc.sync.dma_start(sink_sb[:], sink.ap())
        mp_sb = const.tile([128, NPAT, TT_], f32)
        nc.sync.dma_start(mp_sb[:], mpat.ap())
        id_sb = const.tile([128, 128], bf16)
        nc.sync.dma_start(id_sb[:], idbf.ap())
        ones_col = const.tile([128, 1], bf16)
        nc.vector.memset(ones_col[:], 1.0)
        ones_row = const.tile([1, 128], f32)
        nc.vector.memset(ones_row[:], 1.0)

        # persistent K^T / V accumulation buffers (filled tile-by-tile)
        kT_sb = const.tile([128, B, S_], bf16)
        v_sb = const.tile([128, B, KCB, HD], bf16)

        xT_r = xT.ap().rearrange("(o p) t -> p o t", p=128)

        def rope(dst, src_f, cos_sb, sin_sb, t0):
            """dst[hd,t] = src*cos + rotate_half(src)*sin  (sin sign-folded).

            src_f: [128, TT] bf16 SBUF tile (pre-RoPE projection incl bias).
            """
            ssh = rpool.tile([128, TT_], bf16, tag="ssh")
            # partition rotate-by-64 via SBUF->SBUF DMA
            nc.sync.dma_start(ssh[0:64, :], src_f[64:128, :])
            nc.sync.dma_start(ssh[64:128, :], src_f[0:64, :])
            t1 = rpool.tile([128, TT_], bf16, tag="t1")
            nc.vector.tensor_mul(out=t1[:], in0=src_f[:], in1=cos_sb[:, t0:t0 + TT_])
            t2 = rpool.tile([128, TT_], bf16, tag="t2")
            nc.vector.tensor_mul(out=t2[:], in0=ssh[:], in1=sin_sb[:, t0:t0 + TT_])
            nc.vector.tensor_add(out=dst, in0=t1[:], in1=t2[:])

        for tt in range(NT):
            b = tt // NT_B
            t0 = (tt % NT_B) * TT_
            g0 = tt * TT_             # global token offset

            # ---- stream x^T block for this t-tile ----
            xq = []
            for qq in range(NDQ):
                xt_q = xpool.tile([128, DQ, TT_], bf16, tag="xt")
                nc.sync.dma_start(
                    xt_q[:], xT_r[:, qq * DQ:(qq + 1) * DQ, g0:g0 + TT_]
                )
                xq.append(xt_q)

            def proj_matmuls(ps, w_sb, esl):
                for dc in range(DC):
                    nc.tensor.matmul(
                        ps[:],
                        lhsT=w_sb[:, dc, esl],
                        rhs=xq[dc // DQ][:, dc % DQ, :],
                        start=(dc == 0),
                        stop=(dc == DC - 1),
                    )

            # ---- Q projection + RoPE ----
            q_blk = qpool.tile([128, EH, TT_], bf16)
            for e in range(EH):
                ps_q = psum.tile([128, TT_], f32, tag="ps")
                proj_matmuls(ps_q, wq_sb, slice(e * 128, (e + 1) * 128))
                qf = rpool.tile([128, TT_], bf16, tag="projf")
                nc.scalar.add(qf[:], ps_q[:], bq_sb[:, e:e + 1])
                rope(q_blk[:, e, :], qf, cosq_sb, sinq_sb, t0)

            # ---- K projection + RoPE (pre-scaled trig) ----
            ps_k = psum.tile([128, TT_], f32, tag="ps")
            proj_matmuls(ps_k, wk_sb, slice(0, HD))
            kf = rpool.tile([128, TT_], bf16, tag="projf")
            nc.scalar.add(kf[:], ps_k[:], bk_sb[:, 0:1])
            rope(kT_sb[:, b, t0:t0 + TT_], kf, cosk_sb, sink_sb, t0)

            # ---- V projection (V^T then transpose to natural [k, hd]) ----
            ps_v = psum.tile([128, TT_], f32, tag="ps")
            proj_matmuls(ps_v, wv_sb, slice(0, HD))
            vf = rpool.tile([128, TT_], bf16, tag="projf")
            nc.scalar.add(vf[:], ps_v[:], bv_sb[:, 0:1])
            for j in range(TT_ // 128):
                ps_t = psum.tile([128, 128], bf16, tag="ps")
                nc.tensor.transpose(ps_t[:], vf[:, j * 128:(j + 1) * 128], id_sb[:])
                nc.vector.tensor_copy(
                    out=v_sb[:, b, t0 // 128 + j, :], in_=ps_t[:]
                )

            # ---- attention for this query tile ----
            # Chunk order: interleave the 4 diagonal (masked) chunks between
            # full chunks so their extra DVE mask-add overlaps PE streaming.
            # The chunk loop is software-pipelined depth 2: AV/den matmuls for
            # chunk i issue after S^T/exp of chunk i+2, so the PE never waits
            # on ScalarE's exp.
            nkc = (t0 + TT_) // 128
            diag = list(range(t0 // 128, nkc))
            full = list(range(t0 // 128))
            order = []
            if full:
                stride = max(1, len(full) // len(diag))
                fi = 0
                for d_ in diag:
                    order.append(d_)
                    order.extend(full[fi:fi + stride])
                    fi += stride
                order.extend(full[fi:])
            else:
                order = diag
            assert sorted(order) == list(range(nkc))

            att = apool.tile([128, EH, TT_], bf16)
            pending = None  # (h, av, rec) epilogue deferred one head

            def epilogue(h_, av_, rec_):
                # broadcast 1/den across partitions (rank-1 matmul) and
                # normalize; rec_ was computed during the next head's chunk
                # loop so the bcast matmul never stalls the PE.
                bc = psum.tile([128, TT_], f32, tag="ps")
                nc.tensor.matmul(
                    bc[:], lhsT=ones_row[:], rhs=rec_[:], start=True, stop=True
                )
                avs = spool.tile([128, TT_], f32, tag="avs", bufs=1)
                nc.vector.tensor_copy(out=avs[:], in_=av_[:])
                nc.vector.tensor_mul(out=att[:, h_, :], in0=avs[:], in1=bc[:])

            for h in range(EH):
                av = psum.tile([128, TT_], f32, tag="ps")
                den = psum.tile([1, TT_], f32, tag="ps")
                D_AV, D_DEN = 3, 5   # exp->AV slack 3 chunks; den trails AV
                pts = {}

                def do_av(j, av=av):
                    nc.tensor.matmul(
                        av[:], lhsT=v_sb[:, b, order[j], :], rhs=pts[j][:],
                        start=(j == 0), stop=(j == nkc - 1),
                    )

                def do_den(j, den=den):
                    nc.tensor.matmul(
                        den[:], lhsT=ones_col[:], rhs=pts.pop(j)[:],
                        start=(j == 0), stop=(j == nkc - 1),
                    )

                for i, kc in enumerate(order):
                    st = psum.tile([128, TT_], f32, tag="ps")
                    nc.tensor.matmul(
                        st[:],
                        lhsT=kT_sb[:, b, kc * 128:(kc + 1) * 128],
                        rhs=q_blk[:, h, :],
                        start=True,
                        stop=True,
                    )
                    d = kc * 128 - t0
                    if d >= 0:  # diagonal band: apply causal mask pattern
                        nc.vector.tensor_add(
                            out=st[:], in0=st[:], in1=mp_sb[:, d // 128, :]
                        )
                    pt = ppool.tile([128, TT_], bf16, tag="pt")
                    nc.scalar.activation(pt[:], st[:], AF.Exp)
                    pts[i] = pt
                    if i >= D_AV:
                        do_av(i - D_AV)
                    if i >= D_DEN:
                        do_den(i - D_DEN)
                for j in range(max(0, nkc - D_AV), nkc):
                    do_av(j)
                for j in range(max(0, nkc - D_DEN), nkc):
                    do_den(j)

                rec = spool.tile([1, TT_], f32, tag="rec")
                nc.vector.reciprocal(rec[:], den[:])
                if pending is not None:
                    epilogue(*pending)
                pending = (h, av, rec)
            epilogue(*pending)

            # ---- output projection (partial, transposed) ----
            # PSUM->SBUF staging on DVE (keeps ScalarE free for next tile's
            # exps); partial writes on the ACT HWDGE queue so the SP queue
            # only carries latency-critical loads.
            for dt in range(DC):
                po = psum.tile([128, TT_], f32, tag="ps")
                for c in range(EH):
                    nc.tensor.matmul(
                        po[:],
                        lhsT=wo_sb[:, c, dt * 128:(dt + 1) * 128],
                        rhs=att[:, c, :],
                        start=(c == 0),
                        stop=(c == EH - 1),
                    )
                osb = opool.tile([128, TT_], f32, tag="osb")
                if dt % 2 == 0:
                    nc.scalar.copy(osb[:], po[:])
                else:
                    nc.vector.tensor_copy(out=osb[:], in_=po[:])
                if host_reduce:
                    nc.scalar.dma_start(
                        y.ap()[dt * 128:(dt + 1) * 128, g0:g0 + TT_], osb[:]
                    )
                else:
                    nc.scalar.dma_start(
                        partials[tt].ap()[dt * 128:(dt + 1) * 128, :], osb[:]
                    )

            if not host_reduce:
                # ---- chunked reduce-scatter + final copy ----
                import concourse.mybir as mybir_  # noqa: PLC0415
                nc.gpsimd.collective_compute(
                    "ReduceScatter",
                    mybir_.AluOpType.add,
                    replica_groups=[list(range(NCORES))],
                    ins=[partials[tt].ap().opt()],
                    outs=[rsouts[tt].ap().opt()],
                )
                nc.gpsimd.dma_start(y.ap()[:, g0:g0 + TT_], rsouts[tt].ap())

    nc.finalize()
    return nc


def _prep_in_maps(x, cos, sin, Wq, bq, Wk, bk, Wv, bv, Wo, mask, D, S_, TT_):
    """Host-side sharding/prep: transpose+cast per-core operand slices."""
    NPAT = TT_ // 128
    scaling = np.float32(1.0 / np.sqrt(HD))

    xT = np.ascontiguousarray(
        x.reshape(B * S_, D).T.astype(BF16)
    )  # [D, B*S]
    cosT = cos.T.astype(np.float32)            # [HD, S]
    sinT = sin.T.astype(np.float32)
    sgn = np.ones((HD, 1), np.float32)
    sgn[: HD // 2] = -1.0
    cosq_h = np.ascontiguousarray(cosT.astype(BF16))
    sinq_h = np.ascontiguousarray((sinT * sgn).astype(BF16))
    cosk_h = np.ascontiguousarray((cosT * scaling).astype(BF16))
    sink_h = np.ascontiguousarray((sinT * sgn * scaling).astype(BF16))

    # causal mask diagonal patterns from the mask input:
    # mpat[p, i, f] = clip(mask[f, i*128 + p], -100, 0)
    m = np.asarray(mask).reshape(np.asarray(mask).shape[-2], -1)[:TT_, :TT_]
    mp = np.clip(m.T, -100.0, 0.0).astype(np.float32)        # [k, t]
    mpat = np.ascontiguousarray(
        mp.reshape(NPAT, 128, TT_).transpose(1, 0, 2)
    )  # [p, i, f]

    idbf = np.eye(128, dtype=BF16)

    in_maps = []
    for c in range(NCORES):
        e0 = c * E
        k0 = c * HD
        in_maps.append({
            "xT": xT,
            "wqT": np.ascontiguousarray(Wq[e0:e0 + E].T.astype(BF16)),
            "wkT": np.ascontiguousarray(Wk[k0:k0 + HD].T.astype(BF16)),
            "wvT": np.ascontiguousarray(Wv[k0:k0 + HD].T.astype(BF16)),
            "woT": np.ascontiguousarray(Wo[:, e0:e0 + E].T.astype(BF16)),
            "bqc": np.ascontiguousarray(
                bq[e0:e0 + E].reshape(EH, 128).T.astype(np.float32)
            ),
            "bkc": bk[k0:k0 + HD].reshape(HD, 1).astype(np.float32),
            "bvc": bv[k0:k0 + HD].reshape(HD, 1).astype(np.float32),
            "cosq": cosq_h,
            "sinq": sinq_h,
            "cosk": cosk_h,
            "sink": sink_h,
            "mpat": mpat,
            "idbf": idbf,
        })
    return in_maps


_NC_CACHE = {}


def _install_ntff_hook():
    """Register the axon NTFF profiling hook that this image's antenv lacks
    (used only when KERNEL_TRACE=1; grading runs without it)."""
    import types

    try:
        import antenv
        if "antenv.axon_hooks" not in sys.modules:
            mod = types.ModuleType("antenv.axon_hooks")
            state = {"hook": None}
            mod.set_axon_ntff_profile_hook = lambda h: state.__setitem__("hook", h)
            mod.get_axon_ntff_profile_hook = lambda: state["hook"]
            sys.modules["antenv.axon_hooks"] = mod
            antenv.axon_hooks = mod
        import antenv.axon_hooks as ah
        if ah.get_axon_ntff_profile_hook() is None:
            from trn_agent_boot.trn_boot import _ntff_profile_via_ctypes
            ah.set_axon_ntff_profile_hook(
                _ntff_profile_via_ctypes("/opt/axon/libaxon_pjrt.so")
            )
        return True
    except Exception as e:  # pragma: no cover - best effort
        print(f"NTFF hook install failed ({e}); running without trace")
        return False


def kernel(**inputs):
    x = np.asarray(inputs["x"], np.float32)
    cos = np.asarray(inputs["cos"], np.float32)
    sin = np.asarray(inputs["sin"], np.float32)
    Wq = np.asarray(inputs["Wq"], np.float32)
    bq = np.asarray(inputs["bq"], np.float32)
    Wk = np.asarray(inputs["Wk"], np.float32)
    bk = np.asarray(inputs["bk"], np.float32)
    Wv = np.asarray(inputs["Wv"], np.float32)
    bv = np.asarray(inputs["bv"], np.float32)
    Wo = np.asarray(inputs["Wo"], np.float32)
    mask = np.asarray(inputs["mask"], np.float32)
    start_pos = int(inputs.get("start_pos", 0))
    assert start_pos == 0, "kernel specialized for start_pos=0"

    from concourse.bass_utils import run_bass_kernel_spmd

    key = (DIM, S, TT)
    if key not in _NC_CACHE:
        _NC_CACHE[key] = build_module(DIM, S, TT)
    nc = _NC_CACHE[key]

    in_maps = _prep_in_maps(x, cos, sin, Wq, bq, Wk, bk, Wv, bv, Wo, mask,
                            DIM, S, TT)

    trace = bool(int(os.environ.get("KERNEL_TRACE", "0")))
    if trace:
        trace = _install_ntff_hook()
    res = run_bass_kernel_spmd(
        nc, in_maps, core_ids=list(range(NCORES)), trace=trace,
        trace_cores=list(range(NCORES)) if trace else None,
    )
    if trace:
        kernel.last_results = res

    if HOST_REDUCE:
        fullT = res.results[0]["y"].astype(np.float64)
        for c in range(1, NCORES):
            fullT += res.results[c]["y"]
        fullT = fullT.astype(np.float32)
    else:
        fullT = np.concatenate([res.results[c]["y"] for c in range(NCORES)],
                               axis=0)
    return np.ascontiguousarray(fullT.T).reshape(B, S, DIM).astype(np.float32)
